# revision 1
# baseline (speedup 1.0000x reference)
"""Multi-head causal attention (B=2, S=2048, DIM=2048, H=16, HD=128) with RoPE,
distributed over 8 Trainium2 NeuronCores.

Sharding: data-parallel over batch (2) x tensor-parallel over head groups (4):
core = b*4 + g handles batch b, heads [4g, 4g+4). Each core computes
Q/K/V projections for its head group (bf16 matmuls, fp32 psum), applies RoPE,
runs causal flash-style attention entirely in "d-major" layouts (no on-device
transposes), applies the output projection rows for its heads, and returns a
partial (S, DIM) output. Host sums the 4 partials per batch (row-parallel wo).

Layout tricks:
  - x is fed pre-transposed (xT, dim-major): serves as lhsT for V and as the
    moving operand for Q^T/K^T, so projections directly produce d-major Q^T/K^T.
  - RoPE in d-major: rot = raw*C + (P_swap @ raw)*S_signed, where the
    pair-swap is one 128x128 matmul against a constant permutation matrix.
  - Scores are computed transposed (S^T tiles, j on partitions), softmax is
    max-free (scores ~ N(0,1): exp never overflows), row sums via a
    ones-column matmul, reciprocal via a tiny DMA-transpose round trip, and
    the 1/L broadcast via a gpsimd partition_broadcast.
  - P^T tiles feed P@V directly; attention output lands d-major (O^T), which
    is exactly the stationary operand the output projection needs.
"""

import numpy as np
import ml_dtypes

import concourse.bacc as bacc
import concourse.mybir as mybir
import concourse.tile as tile
from concourse.bass_utils import run_bass_kernel_spmd

B, S, DIM, H, HD = 2, 2048, 2048, 16, 128
NCORES = 8
GROUPS = 4               # head groups (tensor-parallel)
HPC = H // GROUPS        # 4 heads per core
GD = HPC * HD            # 512 dims per group
NKT = DIM // 128         # 16 contraction tiles
NSB = S // 512           # 4 s blocks
NIB = S // 512           # 4 i blocks
F32 = mybir.dt.float32
BF16 = mybir.dt.bfloat16
BF = ml_dtypes.bfloat16
NEG = -1e9

_CACHE = {}


def _build():
    nc = bacc.Bacc("TRN2", target_bir_lowering=False, debug=False,
                   num_devices=NCORES)
    xT = nc.dram_tensor("xT", [DIM, S], BF16, kind="ExternalInput").ap()
    wq = nc.dram_tensor("wq", [DIM, GD], BF16, kind="ExternalInput").ap()
    wk = nc.dram_tensor("wk", [DIM, GD], BF16, kind="ExternalInput").ap()
    wv = nc.dram_tensor("wv", [DIM, GD], BF16, kind="ExternalInput").ap()
    wo = nc.dram_tensor("wo", [GD, DIM], BF16, kind="ExternalInput").ap()
    ropeC = nc.dram_tensor("ropeC", [HD, S], BF16, kind="ExternalInput").ap()
    ropeS = nc.dram_tensor("ropeS", [HD, S], BF16, kind="ExternalInput").ap()
    tri = nc.dram_tensor("tri", [128, 128], F32, kind="ExternalInput").ap()
    pmat = nc.dram_tensor("pmat", [128, 128], BF16, kind="ExternalInput").ap()
    out = nc.dram_tensor("out", [S, DIM], F32, kind="ExternalOutput").ap()

    with tile.TileContext(nc) as tc:
        with (
            tc.tile_pool(name="wpool", bufs=NKT) as wpool,
            tc.tile_pool(name="xpool", bufs=NKT) as xpool,
            tc.tile_pool(name="qkpool", bufs=HPC) as qkpool,
            tc.tile_pool(name="vpool", bufs=S // 128) as vpool,
            tc.tile_pool(name="otpool", bufs=HPC * NIB) as otpool,
            tc.tile_pool(name="wopool", bufs=HPC * 4) as wopool,
            tc.tile_pool(name="cpool", bufs=1) as cpool,
            tc.tile_pool(name="stage", bufs=3) as stage,
            tc.tile_pool(name="tpool", bufs=2) as tpool,
            tc.tile_pool(name="ptpool", bufs=4) as ptpool,
            tc.tile_pool(name="lpool", bufs=3) as lpool,
            tc.tile_pool(name="opool", bufs=3) as opool,
            tc.tile_pool(name="ps_mm", bufs=3, space="PSUM") as ps_mm,
            tc.tile_pool(name="ps_st", bufs=2, space="PSUM") as ps_st,
            tc.tile_pool(name="ps_acc", bufs=3, space="PSUM") as ps_acc,
        ):
            # ---- weights / constants: emission order = DMA priority.
            # First x-strip (sb=0) + wq interleaved so Q-proj starts asap;
            # then wk, wv; consts; wo last (phase C only).
            wq_t, wk_t, wv_t = [], [], []
            xt0 = []
            for kt in range(NKT):
                sl = slice(kt * 128, (kt + 1) * 128)
                t = xpool.tile([128, 512], BF16, tag="xt", name=f"xt0_{kt}")
                nc.sync.dma_start(t[:], xT[sl, 0:512]); xt0.append(t)
                t = wpool.tile([128, GD], BF16, tag="wq")
                nc.sync.dma_start(t[:], wq[sl, :]); wq_t.append(t)
            ropeC_t = cpool.tile([HD, S], BF16, tag="ropeC")
            nc.sync.dma_start(ropeC_t[:], ropeC[:, :])
            ropeS_t = cpool.tile([HD, S], BF16, tag="ropeS")
            nc.sync.dma_start(ropeS_t[:], ropeS[:, :])
            tri_t = cpool.tile([128, 128], F32, tag="tri")
            nc.sync.dma_start(tri_t[:], tri[:, :])
            pmat_t = cpool.tile([128, 128], BF16, tag="pmat")
            nc.sync.dma_start(pmat_t[:], pmat[:, :])
            ones_col = cpool.tile([128, 1], BF16, tag="ones_col")
            nc.vector.memset(ones_col[:], 1.0)
            for kt in range(NKT):
                sl = slice(kt * 128, (kt + 1) * 128)
                t = wpool.tile([128, GD], BF16, tag="wk")
                nc.sync.dma_start(t[:], wk[sl, :]); wk_t.append(t)
            for kt in range(NKT):
                sl = slice(kt * 128, (kt + 1) * 128)
                t = wpool.tile([128, GD], BF16, tag="wv")
                nc.sync.dma_start(t[:], wv[sl, :]); wv_t.append(t)


            # persistent activations (bf16)
            qt_t = [qkpool.tile([128, S], BF16, tag="qt", name=f"qt{h}") for h in range(HPC)]
            kt_t = [qkpool.tile([128, S], BF16, tag="kt", name=f"ktt{h}") for h in range(HPC)]
            v_t = [vpool.tile([128, GD], BF16, tag="v", name=f"v{st}") for st in range(S // 128)]
            ot_t = {}
            for h in range(HPC):
                for ib in range(NIB):
                    ot_t[(h, ib)] = otpool.tile([128, 512], BF16, tag="ot", name=f"ot{h}_{ib}")

            # ---- phase A: projections + rope ----
            for sb in range(NSB):
                s0 = sb * 512
                if sb == 0:
                    xt = xt0
                else:
                    xt = []
                    for kt in range(NKT):
                        t = xpool.tile([128, 512], BF16, tag="xt",
                                       name=f"xt{sb}_{kt}")
                        nc.sync.dma_start(
                            t[:], xT[kt * 128:(kt + 1) * 128, s0:s0 + 512]
                        )
                        xt.append(t)

                for w_t, dst in ((wq_t, qt_t), (wk_t, kt_t)):
                    for h in range(HPC):
                        pmm = ps_mm.tile([128, 512], F32, tag="mm")
                        for kt in range(NKT):
                            nc.tensor.matmul(
                                pmm[:],
                                w_t[kt][:, h * 128:(h + 1) * 128],
                                xt[kt][:],
                                start=(kt == 0), stop=(kt == NKT - 1),
                            )
                        raw = stage.tile([128, 512], BF16, tag="raw")
                        nc.scalar.copy(raw[:], pmm[:])
                        sw = ps_mm.tile([128, 512], F32, tag="mm")
                        nc.tensor.matmul(sw[:], pmat_t[:], raw[:],
                                         start=True, stop=True)
                        t1 = tpool.tile([128, 512], BF16, tag="t1")
                        nc.vector.tensor_mul(t1[:], raw[:],
                                             ropeC_t[:, s0:s0 + 512])
                        t2 = tpool.tile([128, 512], BF16, tag="t2")
                        nc.vector.tensor_mul(t2[:], sw[:],
                                             ropeS_t[:, s0:s0 + 512])
                        nc.vector.tensor_add(dst[h][:, s0:s0 + 512],
                                             t1[:], t2[:])

                for st in range(4):
                    pmm = ps_mm.tile([128, 512], F32, tag="mm")
                    for kt in range(NKT):
                        nc.tensor.matmul(
                            pmm[:],
                            xt[kt][:, st * 128:(st + 1) * 128],
                            wv_t[kt][:],
                            start=(kt == 0), stop=(kt == NKT - 1),
                        )
                    nc.scalar.copy(v_t[sb * 4 + st][:], pmm[:])

            # ---- phase B: attention per (i_block, head) ----
            for ib in range(NIB):
                i0 = ib * 512
                njt = 4 * ib + 4
                for h in range(HPC):
                    l_ps = ps_acc.tile([1, 512], F32, tag="acc")
                    o_ps = ps_acc.tile([128, 512], F32, tag="acc")
                    for jt in range(njt):
                        j0 = jt * 128
                        voff = max(0, j0 - i0)
                        st_ps = ps_st.tile([128, 512], F32, tag="st")
                        nc.tensor.matmul(
                            st_ps[:, voff:512],
                            kt_t[h][:, j0:j0 + 128],
                            qt_t[h][:, i0 + voff:i0 + 512],
                            start=True, stop=True,
                        )
                        if j0 >= i0:
                            nc.vector.tensor_add(
                                st_ps[:, voff:voff + 128],
                                st_ps[:, voff:voff + 128],
                                tri_t[:],
                            )
                        pt = ptpool.tile([128, 512], BF16, tag="pt")
                        nc.scalar.activation(
                            pt[:, voff:512], st_ps[:, voff:512],
                            mybir.ActivationFunctionType.Exp,
                        )
                        nc.tensor.matmul(
                            l_ps[:, voff:512], ones_col[:], pt[:, voff:512],
                            start=(jt == 0), stop=(jt == njt - 1),
                        )
                        nc.tensor.matmul(
                            o_ps[:, voff:512],
                            v_t[jt][:, h * 128:(h + 1) * 128],
                            pt[:, voff:512],
                            start=(jt == 0), stop=(jt == njt - 1),
                        )

                    # normalization: 1/L broadcast, applied to O^T
                    lrow = lpool.tile([1, 512], F32, tag="lrow")
                    nc.scalar.copy(lrow[:], l_ps[:])
                    ltc = lpool.tile([128, 4], F32, tag="ltc")
                    for c in range(4):
                        nc.sync.dma_start(ltc[:, c:c + 1],
                                          lrow[0:1, c * 128:(c + 1) * 128])
                    rlt = lpool.tile([128, 4], F32, tag="rlt")
                    nc.vector.reciprocal(rlt[:], ltc[:])
                    rrow = lpool.tile([1, 512], F32, tag="rrow")
                    for c in range(4):
                        nc.sync.dma_start(rrow[0:1, c * 128:(c + 1) * 128],
                                          rlt[:, c:c + 1])
                    bc = opool.tile([128, 512], F32, tag="bc")
                    nc.gpsimd.partition_broadcast(bc[:], rrow[:], channels=128)
                    nc.vector.tensor_mul(ot_t[(h, ib)][:], o_ps[:], bc[:])

            # ---- phase C: output projection (partial over this head group) ----
            wo_t = {}
            for h in range(HPC):
                for eb in range(4):
                    t = wopool.tile([128, 512], BF16, tag="wo")
                    nc.sync.dma_start(
                        t[:], wo[h * 128:(h + 1) * 128, eb * 512:(eb + 1) * 512]
                    )
                    wo_t[(h, eb)] = t

            for stile in range(S // 128):
                ib, soff = stile // 4, (stile % 4) * 128
                for eb in range(4):
                    pmm = ps_mm.tile([128, 512], F32, tag="mm")
                    for h in range(HPC):
                        nc.tensor.matmul(
                            pmm[:],
                            ot_t[(h, ib)][:, soff:soff + 128],
                            wo_t[(h, eb)][:],
                            start=(h == 0), stop=(h == HPC - 1),
                        )
                    co = stage.tile([128, 512], F32, tag="co")
                    if eb % 2 == 0:
                        nc.scalar.copy(co[:], pmm[:])
                    else:
                        nc.vector.tensor_copy(co[:], pmm[:])
                    nc.sync.dma_start(
                        out[stile * 128:(stile + 1) * 128,
                            eb * 512:(eb + 1) * 512],
                        co[:],
                    )

    nc.compile()
    return nc


def _host_inputs(x, freqs_cos, freqs_sin, wq, wk, wv, wo):
    """Build the 8 per-core input maps (host-side sharding + layout prep)."""
    scale = 1.0 / np.sqrt(HD)
    # rope tables, d-major duplicated/interleaved: C[d,s]=cos[s,d//2];
    # S[2j,s]=-sin[s,j]; S[2j+1,s]=+sin[s,j]
    c = np.asarray(freqs_cos, dtype=np.float32)      # (S, HD/2)
    s = np.asarray(freqs_sin, dtype=np.float32)
    ropeC = np.repeat(c.T, 2, axis=0)                # (HD, S)
    ropeS = np.empty((HD, S), dtype=np.float32)
    ropeS[0::2] = -s.T
    ropeS[1::2] = s.T
    ropeC = ropeC.astype(BF)
    ropeS = ropeS.astype(BF)

    tri = np.where(
        np.arange(128)[:, None] <= np.arange(128)[None, :], 0.0, NEG
    ).astype(np.float32)
    pmat = np.zeros((128, 128), dtype=np.float32)
    idx = np.arange(128)
    pmat[idx, idx ^ 1] = 1.0
    pmat = pmat.astype(BF)

    xT = [np.ascontiguousarray(np.asarray(x[b]).T).astype(BF) for b in range(B)]
    wq = np.asarray(wq, dtype=np.float32)
    wk = np.asarray(wk, dtype=np.float32)
    wv = np.asarray(wv, dtype=np.float32)
    wo = np.asarray(wo, dtype=np.float32)

    in_maps = []
    for core in range(NCORES):
        b, g = core // GROUPS, core % GROUPS
        cols = slice(g * GD, (g + 1) * GD)
        in_maps.append({
            "xT": xT[b],
            "wq": np.ascontiguousarray(wq[:, cols] * scale).astype(BF),
            "wk": np.ascontiguousarray(wk[:, cols]).astype(BF),
            "wv": np.ascontiguousarray(wv[:, cols]).astype(BF),
            "wo": np.ascontiguousarray(wo[cols, :]).astype(BF),
            "ropeC": ropeC,
            "ropeS": ropeS,
            "tri": tri,
            "pmat": pmat,
        })
    return in_maps


def _get_nc():
    if "nc" not in _CACHE:
        _CACHE["nc"] = _build()
    return _CACHE["nc"]


def run(inputs, trace=False, tmpdir=None):
    """Run on hardware; returns (full_output, BassKernelResults)."""
    nc = _get_nc()
    in_maps = _host_inputs(
        inputs["x"], inputs["freqs_cos"], inputs["freqs_sin"],
        inputs["wq"], inputs["wk"], inputs["wv"], inputs["wo"],
    )
    res = run_bass_kernel_spmd(
        nc, in_maps, core_ids=list(range(NCORES)), trace=trace, tmpdir=tmpdir
    )
    outs = [np.asarray(res.results[c]["out"], dtype=np.float32)
            for c in range(NCORES)]
    full = np.stack(
        [sum(outs[b * GROUPS + g] for g in range(GROUPS)) for b in range(B)],
        axis=0,
    )
    return full, res


def kernel(**inputs) -> np.ndarray:
    full, _ = run(inputs, trace=False)
    return full

